# revision 8
# baseline (speedup 1.0000x reference)
"""u_dot_v edge scorer for Trainium2 (Bass/Tile), 8-core edge-parallel.

score[e] = dot(h[src[e]], h[dst[e]])  for 1.6M edges, h = [100000, 64] f32.

Strategy: shard edges across the 8 NeuronCores (200K each); h replicated in
each core's DRAM. The fast on-device gather is the SWDGE `dma_gather` ucode,
which takes int16 indices — so h is viewed as 4 chunks of <=32768 rows, and
the host buckets each core's edges into 16 (src_chunk, dst_chunk) groups.
Within a group, one dma_gather call fetches up to B_MAX h-rows per endpoint
from that chunk's slice; DVE multiplies the two gathered tiles and reduces
each 64-feature group to a score. Scores accumulate in a resident SBUF tile
(in bucketed order) and are written back with one DMA; the host un-permutes.

Group buffers have static sizes (mean + 8 sigma of the multinomial bucket
count, rounded to 128); the host pads with index 0 and, in the astronomically
unlikely overflow case, computes the spilled edges in numpy.
"""

import math
import time
from itertools import product

import numpy as np

import concourse.bacc as bacc
import concourse.mybir as mybir
import concourse.tile_sem_assignment as _tsa
from concourse.tile_scheduler import DMAInst as _DMAInst
from concourse.bass_utils import run_bass_kernel_spmd
from concourse.tile import TileContext

# Tile round-robins Pool DMAs over the 8 DMASW sem lanes in scheduled order,
# which desyncs from our per-call SWDGE queue assignment (a DMASW sem lane
# must only ever be updated from one SWDGE queue). Partition the lanes
# per-queue instead: queue q owns lanes {2q, 2q+1}.
if not getattr(_tsa, "_queue_aware_lanes", False):
    _orig_assign_tick = _tsa.TileClockTick._assign_tick

    def _assign_tick_qaware(self, inst):
        qn = getattr(inst, "queue_num", None)
        if (qn is not None and inst.engine == mybir.EngineType.Pool
                and isinstance(inst, _DMAInst)):
            if not hasattr(self, "_q_lane_ctr"):
                self._q_lane_ctr = {}
            c = self._q_lane_ctr.get(qn, 0)
            self._q_lane_ctr[qn] = c + 1
            self.next_sw_dma_idx = 2 * qn + (c % 2)
        return _orig_assign_tick(self, inst)

    _tsa.TileClockTick._assign_tick = _assign_tick_qaware
    _tsa._queue_aware_lanes = True

N_CORES = 8
N_NODES = 100000
D = 64
E_TOTAL = 1600000
E_CORE = E_TOTAL // N_CORES  # 200000
P = 128

CHUNK = 32768
N_CHUNKS = 4
B_MAX = 6144  # max rows per dma_gather call
SINGLE_PACKET = False  # single_packet=True overflows the SWDGE packet above ~1-2K rows/call
MARGIN_SIG = 5.0  # overflow ~3e-7/group; host fallback computes spills
SORT_MODE = "none"  # "z" | "src" | "none" (ordering showed no HW effect)


def chunk_rows(n_nodes, chunk):
    return [min(chunk, n_nodes - c * chunk) for c in range(N_CHUNKS)]


def group_sizes(e_core, n_nodes, chunk):
    """Static per-(src_chunk, dst_chunk) buffer sizes, multiples of 128."""
    rows = chunk_rows(n_nodes, chunk)
    probs = [r / n_nodes for r in rows]
    sizes = []
    for cs, cd in product(range(N_CHUNKS), range(N_CHUNKS)):
        pg = probs[cs] * probs[cd]
        mean = e_core * pg
        sig = math.sqrt(e_core * pg * (1.0 - pg))
        s = int(math.ceil((mean + MARGIN_SIG * sig) / 128.0) * 128)
        sizes.append(max(s, 128))
    return sizes


def split_blocks(n, b_max):
    """Split n (multiple of 128) into pieces <= b_max, each a multiple of 128."""
    out = []
    while n > 0:
        b = min(n, b_max)
        out.append(b)
        n -= b
    return out


def build_nc(n_nodes=N_NODES, chunk=CHUNK, e_core=E_CORE, b_max=B_MAX,
             n_repeat=1, skip_gather=False, skip_compute=False,
             elem_mult=1, n_queues=4, bufs=7, dma_scratch=32768,
             stream_idx=True, idx_bufs=3):
    rows = chunk_rows(n_nodes, chunk)
    sizes = group_sizes(e_core, n_nodes, chunk)
    s_tot = sum(sizes)
    n_cols = s_tot // P

    nc = bacc.Bacc("TRN2", target_bir_lowering=False, debug=False,
                   num_swdge_queues=n_queues,
                   dynamic_dma_scratch_size=dma_scratch)
    em = elem_mult
    h = nc.declare_dram_parameter("h", [n_nodes, D], mybir.dt.float32,
                                  isOutput=False)
    sidx = nc.declare_dram_parameter("sidx", [P, s_tot // 16], mybir.dt.int16,
                                     isOutput=False)
    didx = nc.declare_dram_parameter("didx", [P, s_tot // 16], mybir.dt.int16,
                                     isOutput=False)
    scores = nc.declare_dram_parameter("scores", [P, n_cols],
                                       mybir.dt.float32, isOutput=True)
    cnts = nc.declare_dram_parameter("cnts", [P, 16], mybir.dt.int32,
                                     isOutput=False)

    with TileContext(nc) as tc:
        with (
            tc.tile_pool(name="idx", bufs=1) as idx_pool,
            tc.tile_pool(name="idxs", bufs=idx_bufs) as ispool,
            tc.tile_pool(name="gath", bufs=bufs) as gpool,
            tc.tile_pool(name="score", bufs=1) as spool,
        ):
            if not stream_idx:
                sidx_t = idx_pool.tile([P, s_tot // 16], mybir.dt.int16)
                didx_t = idx_pool.tile([P, s_tot // 16], mybir.dt.int16)
                nc.sync.dma_start(out=sidx_t[:], in_=sidx.ap())
                nc.sync.dma_start(out=didx_t[:], in_=didx.ap())
            cnt_t = idx_pool.tile([P, 16], mybir.dt.int32)
            nc.sync.dma_start(out=cnt_t[:], in_=cnts.ap())

            # Per-group valid count for each group's LAST gather call; pads
            # there are -1 (descriptor generation skips them).
            cnt_regs = []
            for g in range(16):
                r = nc.alloc_register(mybir.EngineType.Pool, name=f"cnt{g}")
                nc.gpsimd.reg_load(r, cnt_t[0:1, g:g + 1])
                cnt_regs.append(r)

            scores_t = spool.tile([P, n_cols], mybir.dt.float32)

            qctr = [0]

            def next_q():
                q = qctr[0] % n_queues
                qctr[0] += 1
                return q

            for _rep in range(n_repeat):
                off = 0
                for g, (cs, cd) in enumerate(product(range(N_CHUNKS),
                                                     range(N_CHUNKS))):
                    h_s = h.ap()[cs * chunk: cs * chunk + rows[cs], :]
                    h_d = h.ap()[cd * chunk: cd * chunk + rows[cd], :]
                    if stream_idx:
                        sidx_t = ispool.tile([P, sizes[g] // 16],
                                             mybir.dt.int16, tag="si")
                        didx_t = ispool.tile([P, sizes[g] // 16],
                                             mybir.dt.int16, tag="di")
                        nc.sync.dma_start(
                            out=sidx_t[:],
                            in_=sidx.ap()[:, off // 16:(off + sizes[g]) // 16])
                        nc.sync.dma_start(
                            out=didx_t[:],
                            in_=didx.ap()[:, off // 16:(off + sizes[g]) // 16])
                        goff = off
                    blocks = split_blocks(sizes[g], b_max)
                    for bi, n in enumerate(blocks):
                        is_last = bi == len(blocks) - 1
                        nreg = cnt_regs[g] if is_last else n
                        nb = n // P
                        hs = gpool.tile([P, nb * D], mybir.dt.float32,
                                        tag="hs")
                        hd = gpool.tile([P, nb * D], mybir.dt.float32,
                                        tag="hd")
                        if not skip_gather:
                            if em > 1:
                                hs2 = gpool.tile([P, nb * D * em],
                                                 mybir.dt.float32, tag="hs2")
                                hd2 = gpool.tile([P, nb * D * em],
                                                 mybir.dt.float32, tag="hd2")
                            g_outs = ((hs, hd) if em == 1 else (hs2, hd2))
                            nc.gpsimd.dma_gather(
                                out_ap=g_outs[0][:].rearrange(
                                    "p (k d) -> p k d", d=D * em),
                                in_ap=(h_s if em == 1 else
                                       h.ap().rearrange(
                                           "(r t) d -> r (t d)", t=em)
                                       [cs * chunk // em:
                                        (cs * chunk + rows[cs]) // em, :]),
                                idxs_ap=sidx_t[:, (off - goff) // 16:
                                               (off - goff + n) // 16]
                                if stream_idx else
                                sidx_t[:, off // 16: (off + n) // 16],
                                num_idxs=n,
                                num_idxs_reg=nreg,
                                elem_size=D * em,
                                single_packet=SINGLE_PACKET,
                                queue_num=next_q(),
                            )
                            nc.gpsimd.dma_gather(
                                out_ap=g_outs[1][:].rearrange(
                                    "p (k d) -> p k d", d=D * em),
                                in_ap=(h_d if em == 1 else
                                       h.ap().rearrange(
                                           "(r t) d -> r (t d)", t=em)
                                       [cd * chunk // em:
                                        (cd * chunk + rows[cd]) // em, :]),
                                idxs_ap=didx_t[:, (off - goff) // 16:
                                               (off - goff + n) // 16]
                                if stream_idx else
                                didx_t[:, off // 16: (off + n) // 16],
                                num_idxs=n,
                                num_idxs_reg=nreg,
                                elem_size=D * em,
                                single_packet=SINGLE_PACKET,
                                queue_num=next_q(),
                            )
                        if not skip_compute:
                            nc.vector.tensor_mul(out=hs[:], in0=hs[:],
                                                 in1=hd[:])
                            nc.vector.tensor_reduce(
                                out=scores_t[:, off // P: off // P + nb],
                                in_=hs[:].rearrange("p (k d) -> p k d", d=D),
                                axis=mybir.AxisListType.X,
                                op=mybir.AluOpType.add,
                            )
                        off += n

            if not skip_compute:
                nc.sync.dma_start(out=scores.ap(), in_=scores_t[:])
    nc.finalize()
    return nc


def shard_core(s, d, n_nodes, chunk, sizes, b_max=B_MAX):
    """Bucket one core's edges into the 16 static group buffers.

    Returns (src16_wrapped, dst16_wrapped, gather_pos, edge_ids, ov_ids)
    where gather_pos[i] is the bucketed position whose score belongs to
    edge edge_ids[i], and ov_ids are edges that overflowed (host-computed).
    """
    s_tot = sum(sizes)
    cs = s // chunk
    cd = d // chunk
    gkey = (cs * N_CHUNKS + cd).astype(np.int64)

    # Z-order (Morton) sort within each group: consecutive edges cluster in
    # (src_row, dst_row) space, so both gathers hit HBM pages in runs
    # instead of fully random 256B reads.
    def _spread(x):
        x = x.astype(np.int64)
        x = (x | (x << 8)) & 0x00FF00FF
        x = (x | (x << 4)) & 0x0F0F0F0F
        x = (x | (x << 2)) & 0x33333333
        x = (x | (x << 1)) & 0x55555555
        return x

    sl = s - cs * chunk
    dl = d - cd * chunk
    if SORT_MODE == "z":
        sub = _spread(sl) | (_spread(dl) << 1)
    elif SORT_MODE == "src":
        sub = sl.astype(np.int64) << 15 | dl
    else:
        sub = np.zeros_like(gkey)
    order = np.argsort((gkey << 32) | sub, kind="stable")
    counts = np.bincount(gkey, minlength=16)

    src16 = np.zeros(s_tot, dtype=np.int16)
    dst16 = np.zeros(s_tot, dtype=np.int16)
    cnts = np.zeros(16, dtype=np.int32)
    gather_pos = []
    edge_ids = []
    ov_ids = []
    base = 0
    gstart = 0
    for g in range(16):
        cnt = int(counts[g])
        take = min(cnt, sizes[g])
        ids = order[gstart: gstart + take]
        src16[base: base + take] = (s[ids] -
                                    (g // N_CHUNKS) * chunk).astype(np.int16)
        dst16[base: base + take] = (d[ids] -
                                    (g % N_CHUNKS) * chunk).astype(np.int16)
        gather_pos.append(np.arange(base, base + take, dtype=np.int64))
        edge_ids.append(ids)
        if cnt > take:
            ov_ids.append(order[gstart + take: gstart + cnt])
        # Last gather call of the group covers the final `last_n` slots:
        # pads there become -1 (generation-skipped); the register count is
        # the number of valid entries in that window (>=1 so the ucode
        # always has a non-negative tail anchor).
        last_n = split_blocks(sizes[g], b_max)[-1]
        win = base + sizes[g] - last_n
        valid_in_win = max(take - (sizes[g] - last_n), 1)
        cnts[g] = valid_in_win
        tail = max(win + valid_in_win, base + take)
        src16[tail: base + sizes[g]] = -1
        dst16[tail: base + sizes[g]] = -1
        gstart += cnt
        base += sizes[g]

    wrap = lambda a: np.tile(np.ascontiguousarray(a.reshape(s_tot // 16, 16).T),
                             (P // 16, 1))
    cnts_w = np.tile(cnts[None, :], (P, 1))
    return (wrap(src16), wrap(dst16), cnts_w,
            np.concatenate(gather_pos), np.concatenate(edge_ids),
            np.concatenate(ov_ids) if ov_ids else np.empty(0, dtype=np.int64))


_NC_CACHE = {}

# Dev knobs: TRACE captures an NTFF profile when available; the
# BassKernelResults of the last run lands in LAST_RESULT.
TRACE = False
LAST_RESULT = None


def _get_nc():
    if "nc" not in _NC_CACHE:
        _NC_CACHE["nc"] = build_nc()
    return _NC_CACHE["nc"]


def kernel(h, src, dst):
    h = np.ascontiguousarray(np.asarray(h), dtype=np.float32)
    src = np.asarray(src).astype(np.int64)
    dst = np.asarray(dst).astype(np.int64)

    sizes = group_sizes(E_CORE, N_NODES, CHUNK)
    in_maps = []
    shards = []
    for c in range(N_CORES):
        lo, hi = c * E_CORE, (c + 1) * E_CORE
        sw, dw, cw, gp, ei, ov = shard_core(src[lo:hi], dst[lo:hi],
                                            N_NODES, CHUNK, sizes)
        in_maps.append({"h": h, "sidx": sw, "didx": dw, "cnts": cw})
        shards.append((gp, ei, ov, lo))

    nc = _get_nc()
    # The axon terminal sporadically reports a transiently desynced mesh;
    # a short backoff and retry recovers it.
    last_exc = None
    for attempt in range(3):
        try:
            res = run_bass_kernel_spmd(nc, in_maps, list(range(N_CORES)),
                                       trace=TRACE)
            break
        except Exception as exc:  # noqa: BLE001
            last_exc = exc
            time.sleep(20 * (attempt + 1))
    else:
        raise last_exc
    global LAST_RESULT
    LAST_RESULT = res

    out = np.empty(E_TOTAL, dtype=np.float32)
    for c in range(N_CORES):
        gp, ei, ov, lo = shards[c]
        sc = res.results[c]["scores"]  # [128, n_cols]
        bucketed = np.ascontiguousarray(sc.T).ravel()  # index = col*128 + p
        out[lo + ei] = bucketed[gp]
        if ov.size:
            out[lo + ov] = np.einsum("ed,ed->e", h[src[lo + ov]],
                                     h[dst[lo + ov]])
    return out

